# revision 49
# baseline (speedup 1.0000x reference)
"""CRF-RNN layer (nn_CrfRnnLayer) as a Bass/Tile SPMD kernel on 8 TRN2 NeuronCores.

Algorithm (matches reference.py):
  N = 112*112 pixels, C = 21 classes, 5 mean-field iterations:
    sm = softmax(Q, axis=classes)
    spatial_out  = (sm @ Ks) / ns      Ks[i,j] = exp(-||p_i-p_j||^2 / (2*3^2))
    bilateral_out= (sm @ Kb) / nb      Kb from (pos/160, rgb/3) features
    Q = u - comp @ (sk @ spatial_out + bk @ bilateral_out)

Sharding: pixel-major 1/8 bands (each core owns 14 image rows = 1568 pixels).

Design notes (vs the fp32 streaming baseline):
  - Everything big runs in bf16/fp8 on the PE (1 cyc/row vs fp32's 4); the
    2e-2 tolerance leaves orders of magnitude of headroom (measured 3e-5).
  - The bilateral kernel slice E [N, 1568] is held RESIDENT in SBUF as
    fp8e4 (153.6KB/partition) - no per-iteration HBM streaming at all.
    It is built once on-device: a K=19 bf16 matmul of hi/lo-split features
    (keeps |d2| error ~1e-2 despite bf16 inputs) + one ACT exp per block.
  - Q is exchanged between cores in pixel-major Q^T [N, C] bf16 layout.
    With the pixel permutation p = 98*r + i for the bilateral blocks
    (1568 = 16*98, so each core owns partition rows 16c..16c+16), BOTH
    per-iteration loads of the gathered tensor are fully contiguous
    (128/112 large descriptors) - the layout-transpose DMAs that dominated
    the baseline (1M+ 4-byte packets) are gone entirely.
  - Spatial filtering is separable: y-pass first (gy2 lhsT), bounce
    [k,(x c)] -> [x,(k c)] through DRAM (42B runs), x-pass emitted
    TRANSPOSED as 3 matmuls producing [(k c), x'] so the class-major
    [c,(k x)] form needed downstream bounces at 448B-run granularity.
  - Per-pixel combine Q^T[p,c'] = u^T + sum_s so42[s,p]*aw42[s,c'] is done
    with 13 pixel-chunk matmuls feeding the bf16 AllGather input directly.
"""

import numpy as np
import ml_dtypes

import concourse.mybir as mybir
import concourse.tile as tile
from concourse import bacc
from concourse.bass import _add_dep_helper
from concourse.bass_utils import run_bass_kernel_spmd


H = 112
W = 112
C = 21
N = H * W
NCORES = 8
YPC = H // NCORES            # 14 image rows per core
COLS = N // NCORES           # 1568 pixels per core
NB = 98                      # bilateral contraction blocks (p = 98*r + i)
KD = 19                      # hi/lo-split feature rows for the d2 matmul
CP = 32                      # lhsT width; cols 21:32 ones
CTS = [(0, 512), (512, 512), (1024, 512), (1536, 32)]
FQ = COLS // 4               # 392: per-col-group F chunk for the bilateral
NCH = 13                     # 1568 = 12*128 + 32 output pixel chunks
NITER = 5
THETA_ALPHA = 160.0
THETA_BETA = 3.0
THETA_GAMMA = 3.0

F32 = mybir.dt.float32
BF16 = mybir.dt.bfloat16
FP8 = mybir.dt.float8e4
U32 = mybir.dt.uint32
EXPF = mybir.ActivationFunctionType.Exp

# Schraudolph constants: host folds v = ASCH*(-0.5 d2) + BSCH into the d2
# matmul (features scaled by sqrt(ASCH), bias rows carry BSCH/2 each).
# ACT blocks recover exp(-0.5 d2) exactly via exp(v*SSCALE + SBIAS); DVE
# blocks clamp+round v to uint32 and bitcast - the float whose bits are
# round(v) approximates exp(-0.5 d2) to ~3% (fp8 is 6% anyway).
ASCH = float(2**23) / float(np.log(2.0))
BSCH = float(127 * (2**23) - 486411)
SSCALE = 1.0 / ASCH
SBIAS = -BSCH / ASCH

_CACHE = {}


def _build_program(reps=1):
    nc = bacc.Bacc("TRN2", target_bir_lowering=False, debug=False, num_devices=NCORES)

    # Chain every PE matmul in emission order (ordering-only deps) so the
    # scheduler keeps same-weights matmuls adjacent -> legalization dedups
    # the LDWEIGHTS instruction for consecutive same-lhsT matmuls.
    _mm_state = {"prev": None}

    def mm(*args, **kwargs):
        inst = nc.tensor.matmul(*args, **kwargs)
        if _mm_state["prev"] is not None:
            _add_dep_helper(inst.ins, _mm_state["prev"].ins, sync=False,
                            reason="pe emission order")
        _mm_state["prev"] = inst
        return inst

    # d2 features, row-tiled pairs: rows 0:19 = even block, rows 32:51 =
    # odd block (ub2) / duplicated slice features (vb2) so two K=19
    # matmuls run CONCURRENTLY in PE row groups 0 and 1.
    ubT = nc.dram_tensor("ubT", [64, (NB // 2) * 128], BF16, kind="ExternalInput")
    vbT_sl = nc.dram_tensor("vbT_sl", [64, COLS], BF16, kind="ExternalInput")
    g2d = nc.dram_tensor("g2d", [W, W], BF16, kind="ExternalInput")
    gy2 = nc.dram_tensor("gy2", [H, YPC], FP8, kind="ExternalInput")
    oh21 = nc.dram_tensor("oh21", [32, 1], F32, kind="ExternalInput")
    u_sl = nc.dram_tensor("u_sl", [C, COLS], F32, kind="ExternalInput")
    uT_d = nc.dram_tensor("uT_d", [128, NCH * C], F32, kind="ExternalInput")
    sm0b_d = nc.dram_tensor("sm0b_d", [128, NB * C], BF16, kind="ExternalInput")
    sm0i_d = nc.dram_tensor("sm0i_d", [H, W * C], BF16, kind="ExternalInput")
    aw54 = nc.dram_tensor("aw54", [54, C], BF16, kind="ExternalInput")
    qt_out = nc.dram_tensor("qt_out", [C, COLS], F32, kind="ExternalOutput")

    with tile.TileContext(nc) as tc:
        with (
            tc.tile_pool(name="const", bufs=1) as cpool,
            tc.tile_pool(name="smx", bufs=1) as smpool,
            tc.tile_pool(name="stream", bufs=2) as stpool,
            tc.tile_pool(name="outp", bufs=1) as opool,
            tc.tile_pool(name="psum", bufs=1, space="PSUM") as pspool,
            tc.tile_pool(name="dram", bufs=1, space="DRAM") as dpool,
        ):
          for _rep in range(reps):
            # ---------------- constants ----------------
            vbT_sb = cpool.tile([64, COLS], BF16, tag="vbT", name=f"vbT_{_rep}")
            nc.sync.dma_start(vbT_sb[:], vbT_sl[:])
            u_sb = cpool.tile([C, COLS], F32, tag="usb", name=f"usb_{_rep}")
            nc.sync.dma_start(u_sb[:], u_sl[:])
            uT_sb = cpool.tile([128, NCH * C], F32, tag="uT", name=f"uT_{_rep}")
            nc.sync.dma_start(uT_sb[:], uT_d[:])
            aw54_sb = cpool.tile([54, C], BF16, tag="aw54", name=f"aw54_{_rep}")
            nc.sync.dma_start(aw54_sb[:], aw54[:])
            gy2_sb = cpool.tile([H, YPC], FP8, tag="gy2", name=f"gy2_{_rep}")
            nc.sync.dma_start(gy2_sb[:], gy2[:])
            oh21_sb = cpool.tile([32, 1], F32, tag="oh21", name=f"oh21_{_rep}")
            nc.sync.dma_start(oh21_sb[:], oh21[:])
            g2d_f = cpool.tile([W, W], BF16, tag="g2df", name=f"g2df_{_rep}")
            nc.sync.dma_start(g2d_f[:], g2d[:])

            invnb_bc = cpool.tile([C, COLS], F32, tag="invnb", name=f"invnb_{_rep}")

            # resident fp8 bilateral kernel slice, [128, 98 blocks, 1568 cols]
            e_res = cpool.tile([128, NB, COLS], FP8, tag="eres", name=f"eres_{_rep}")

            # softmax lhsT [128, block, class+ones]; ones cols written once
            smB = smpool.tile([128, NB, CP], FP8, tag="smB", name=f"smB_{_rep}")
            nc.gpsimd.memset(smB[:, :, C:CP], 1.0)
            # stacked [54, COLS]: spatial rows 0:21, bilateral rows 32:53;
            # rows 21:32 pair with zero aw54 rows - zeroed once. bf16 so the
            # combine matmuls run at 1 cyc/col instead of fp32's 4.
            so54 = cpool.tile([54, COLS], BF16, tag="so54", name=f"so54_{_rep}")
            nc.gpsimd.memset(so54[:], 0.0)

            # DRAM scratch
            qT_sl = dpool.tile([COLS * C], FP8, tag="qtsl", bufs=2,
                               name=f"qtsl_{_rep}")

            # Tiny AllGather fired at program start: absorbs inter-core
            # launch skew during the E-build so the first real gather does
            # not pay a straggler wait.
            sync_in = dpool.tile([32], FP8, tag="syin", name=f"syin_{_rep}")
            sync_out = dpool.tile([NCORES * 32], FP8, tag="syout",
                                  addr_space="Shared", name=f"syout_{_rep}")
            nc.gpsimd.collective_compute(
                "AllGather",
                mybir.AluOpType.bypass,
                replica_groups=[list(range(NCORES))],
                ins=[sync_in[:]],
                outs=[sync_out[:]],
            )

            # ---- iteration-0 inputs (host-computed softmax(u)) ----
            qb0 = smpool.tile([128, NB * C], BF16, tag="qb",
                              name=f"qb0_{_rep}")
            nc.sync.dma_start(qb0[:], sm0b_d[:])
            nc.vector.tensor_copy(
                smB[:, :, 0:C], qb0[:].rearrange("r (i c) -> r i c", c=C)
            )
            smi0 = smpool.tile([H, W * C], BF16, tag="qi",
                               name=f"smi0_{_rep}")
            nc.sync.dma_start(smi0[:], sm0i_d[:])
            smi_i0 = smpool.tile([H, W * C], FP8, tag="smi",
                                 name=f"smi_{_rep}_0")
            nc.vector.tensor_copy(smi_i0[:], smi0[:])

            def spatial_y(smi, it):
                # y-pass TRANSPOSED per class: tT[x, c, k] =
                # sum_y smi[y,x,c] * gy2[y, k] (lhsT = stride-21 slice of
                # smi). 1/ns is folded into gy2/g2d columns host-side.
                tT_ps = pspool.tile([W, 294], F32, tag="tail",
                                    name=f"tT_{_rep}_{it}")
                smi_cx = smi[:].rearrange("y (x c) -> y c x", c=C)
                for c in range(C):
                    mm(tT_ps[:, c * YPC : (c + 1) * YPC], smi_cx[:, c, :],
                       gy2_sb[:], start=True, stop=True)
                tT_sb = smpool.tile([W, 294], BF16, tag="sums",
                                    name=f"tTs_{_rep}_{it}")
                nc.vector.tensor_copy(tT_sb[:], tT_ps[:])
                return tT_sb

            def spatial_x(tT_sb, it):
                # x-pass picks class-major lhsT cols from tT so each of the
                # 14 k-matmuls emits so[c, k, x'] directly into class-major
                # PSUM (128-padded k-chunks keep each dst inside one bank);
                # split over the two d2 regions (7 k-chunks each).
                tT_kc = tT_sb[:].rearrange("x (c k) -> x k c", k=YPC)
                for half in range(2):
                    so_ps = pspool.tile([C, 7 * 128], F32,
                                        tag=("d2a" if half == 0 else "d2b"),
                                        name=f"soT_{_rep}_{it}_{half}")
                    for kk in range(7):
                        k = half * 7 + kk
                        mm(so_ps[:, kk * 128 : kk * 128 + W], tT_kc[:, k, :],
                           g2d_f[:], start=True, stop=True)
                    nc.vector.tensor_copy(
                        so54[0:C, half * 7 * W : (half + 1) * 7 * W].rearrange(
                            "c (k x) -> c k x", x=W
                        ),
                        so_ps[:].rearrange("c (k x) -> c k x", x=128)[:, :, 0:W],
                    )

            # ---------------- E-build ----------
            # ACT-bound exp stream; every 3rd block's exp runs on the DVE
            # instead (Schraudolph bit-trick), cutting the ACT wall ~1/3.
            NBB = 14
            u32s = smpool.tile([128, 1536], U32, tag="u32s",
                               name=f"u32s_{_rep}")
            sbias = cpool.tile([128, 1], F32, tag="sbias", name=f"sbias_{_rep}")
            nc.gpsimd.memset(sbias[:], SBIAS)

            def bilat_blk(i, bl_ps, lhs, pw):
                for g in range(4):
                    mm(
                        bl_ps[32 * g : 32 * g + pw, 0:FQ],
                        lhs,
                        e_res[:, i, FQ * g : FQ * (g + 1)],
                        start=(i == 0),
                        stop=(i == NB - 1),
                        tile_position=(0, 32 * g),
                    )

            for b in range(NB // NBB):
                # even-block tails in the "tail" bank, odd-block tails in the
                # "bil" bank (free during the E-build) - two concurrent
                # row-tiled matmuls must not drain into the same PSUM bank
                tail_ps = pspool.tile([128, 512], F32, tag="tail",
                                      name=f"tail_{_rep}_{b}")
                tato_ps = pspool.tile([128, 512], F32, tag="bil",
                                      name=f"tato_{_rep}_{b}")
                for bq in range(NBB // 2):
                    q = b * (NBB // 2) + bq
                    i0 = 2 * q
                    ub19 = stpool.tile([64, 128], BF16, tag="ub19",
                                       name=f"ub19_{_rep}_{q}")
                    nc.sync.dma_start(ub19[:], ubT[:, q * 128 : (q + 1) * 128])
                    d2a = pspool.tile([128, 1536], F32, tag="d2a",
                                      name=f"d2_{_rep}_{i0}")
                    d2b = pspool.tile([128, 1536], F32, tag="d2b",
                                      name=f"d2_{_rep}_{i0 + 1}")
                    # the even/odd matmuls of each chunk run concurrently in
                    # PE row groups 0 / 1
                    for ci in range(3):
                        mm(d2a[:, ci * 512 : (ci + 1) * 512], ub19[0:KD, :],
                           vbT_sb[0:KD, ci * 512 : (ci + 1) * 512],
                           start=True, stop=True, tile_position=(0, 0))
                        mm(d2b[:, ci * 512 : (ci + 1) * 512], ub19[32:32 + KD, :],
                           vbT_sb[32:32 + KD, ci * 512 : (ci + 1) * 512],
                           start=True, stop=True, tile_position=(32, 0))
                    mm(tail_ps[:, bq * 32 : (bq + 1) * 32],
                       ub19[0:KD, :], vbT_sb[0:KD, 1536:1568],
                       start=True, stop=True, tile_position=(0, 0))
                    mm(tato_ps[:, bq * 32 : (bq + 1) * 32],
                       ub19[32:32 + KD, :], vbT_sb[32:32 + KD, 1536:1568],
                       start=True, stop=True, tile_position=(32, 0))
                    for i, d2_ps in ((i0, d2a), (i0 + 1, d2b)):
                        if i % 3 == 2:
                            nc.vector.tensor_scalar_max(u32s[:], d2_ps[:], 0.0)
                            nc.vector.tensor_copy(
                                e_res[:, i, 0:1536], u32s[:].bitcast(F32)
                            )
                        else:
                            nc.scalar.activation(e_res[:, i, 0:1536], d2_ps[:],
                                                 EXPF, bias=sbias[:],
                                                 scale=SSCALE)
                e_tl = e_res[:, b * NBB : (b + 1) * NBB, 1536:1568].rearrange(
                    "p (q two) c -> p two q c", two=2
                )
                nc.scalar.activation(
                    e_tl[:, 0],
                    tail_ps[:, 0 : 7 * 32].rearrange("p (q c) -> p q c", c=32),
                    EXPF, bias=sbias[:], scale=SSCALE,
                )
                nc.scalar.activation(
                    e_tl[:, 1],
                    tato_ps[:, 0 : 7 * 32].rearrange("p (q c) -> p q c", c=32),
                    EXPF, bias=sbias[:], scale=SSCALE,
                )
                if b == 3:
                    # mid-program resync: absorbs skew accumulated since the
                    # start-of-program sync so the first real gather is cheap
                    sync2_out = dpool.tile([NCORES * 32], FP8, tag="syout2",
                                           addr_space="Shared",
                                           name=f"syout2_{_rep}")
                    nc.gpsimd.collective_compute(
                        "AllGather",
                        mybir.AluOpType.bypass,
                        replica_groups=[list(range(NCORES))],
                        ins=[sync_in[:]],
                        outs=[sync2_out[:]],
                    )
            # ---- it-0 bilateral burst + spatial (interleaved) ----
            bl0_ps = pspool.tile([128, 512], F32, tag="bil",
                                 name=f"bl_{_rep}_0")
            for j in range(NB // 2):
                bilat_blk(j, bl0_ps, smB[:, j, :], CP)
            tT0_sb = spatial_y(smi_i0, 0)
            for j in range(NB // 2, NB):
                bilat_blk(j, bl0_ps, smB[:, j, :], CP)
            spatial_x(tT0_sb, 0)

            # ---------------- iterations ----------------
            for it in range(NITER):
                if it > 0:
                    # softmax arrives pre-computed via the gather (fp8);
                    # load both layouts directly, no per-iteration softmax.
                    qb = smpool.tile([128, NB * C], FP8, tag="qb",
                                     name=f"qb_{_rep}_{it}")
                    nc.sync.dma_start(
                        qb[:], qT_full[:].rearrange("(r q) -> r q", r=128)
                    )
                    smi = smpool.tile([H, W * C], FP8, tag="smi",
                                      name=f"smi_{_rep}_{it}")
                    nc.sync.dma_start(
                        smi[:], qT_full[:].rearrange("(y w) -> y w", y=H)
                    )

                    # bilateral: resident fp8 E, col-tiled x4 (the 4 col
                    # groups run concurrently, each on its own 392-col pixel
                    # chunk); lhsT straight from the gathered qb. Output
                    # [4 groups x 21, FQ] in ONE PSUM bank.
                    bl_ps = pspool.tile([128, 512], F32, tag="bil",
                                        name=f"bl_{_rep}_{it}")
                    qb_i = qb[:].rearrange("r (i c) -> r i c", c=C)
                    # interleave the spatial passes into the bilateral
                    # stream so their DVE casts overlap the PE burst
                    for i in range(NB // 2):
                        bilat_blk(i, bl_ps, qb_i[:, i, :], C)
                    tT_sb = spatial_y(smi, it)
                    for i in range(NB // 2, NB):
                        bilat_blk(i, bl_ps, qb_i[:, i, :], C)
                    spatial_x(tT_sb, it)
                else:
                    bl_ps = bl0_ps

                # ---- iteration 0: build 1/nb broadcast across class partitions
                # (nb sits at PSUM partition 21 - extract to partition 0 with a
                # one-hot matmul since engine APs need 32-aligned partitions)
                if it == 0:
                    # nb sits at partition 21 of each col group; copy each
                    # group's [32, FQ] slab to base 0 and one-hot extract.
                    rnb = smpool.tile([1, COLS], F32, tag="eq", name=f"rnb_{_rep}")
                    for g in range(4):
                        blsb = smpool.tile([32, FQ], F32, tag="eqi",
                                           name=f"blsb_{_rep}_{g}")
                        nc.vector.tensor_copy(
                            blsb[:], bl_ps[32 * g : 32 * g + 32, 0:FQ]
                        )
                        rnb_ps = pspool.tile([1, 512], F32, tag="tail",
                                             name=f"rnbp_{_rep}_{g}")
                        mm(rnb_ps[:, 0:FQ], oh21_sb[:],
                           blsb[:], start=True, stop=True)
                        nc.vector.reciprocal(
                            rnb[:, FQ * g : FQ * (g + 1)], rnb_ps[0:1, 0:FQ]
                        )
                    nc.gpsimd.partition_broadcast(invnb_bc[:], rnb[0:1, :])

                # ---- stacked so42 [42, COLS]: spatial rows 0:21 (written by
                # the x-pass cast above), bilateral rows 32:53
                for g in range(4):
                    nc.vector.tensor_mul(
                        so54[32:53, FQ * g : FQ * (g + 1)],
                        bl_ps[32 * g : 32 * g + C, 0:FQ],
                        invnb_bc[:, FQ * g : FQ * (g + 1)],
                    )

                if it < NITER - 1:
                    # ---- Q^T chunks: qT[p, c'] = u^T + sum_s so42[s,p] aw42[s,c']
                    qT_ps = pspool.tile([128, NCH * C], F32, tag="tail",
                                        name=f"qTp_{_rep}_{it}")
                    for ch in range(NCH):
                        pw = 128 if ch < NCH - 1 else 32
                        mm(
                            qT_ps[0:pw, ch * C : (ch + 1) * C],
                            so54[:, ch * 128 : ch * 128 + pw],
                            aw54_sb[:],
                            start=True,
                            stop=True,
                        )
                    qT_sb = opool.tile([128, NCH * C], BF16, tag="qTsb",
                                       name=f"qTsb_{_rep}_{it}")
                    nc.vector.tensor_add(
                        qT_sb[:, 0 : (NCH - 1) * C],
                        qT_ps[:, 0 : (NCH - 1) * C],
                        uT_sb[:, 0 : (NCH - 1) * C],
                    )
                    nc.vector.tensor_add(
                        qT_sb[0:32, (NCH - 1) * C : NCH * C],
                        qT_ps[0:32, (NCH - 1) * C : NCH * C],
                        uT_sb[0:32, (NCH - 1) * C : NCH * C],
                    )
                    # softmax over classes of OUR slice only (8x less work
                    # than softmaxing the gathered full tensor next iter)
                    eqT = opool.tile([128, NCH * C], BF16, tag="eqT",
                                     name=f"eqT_{_rep}_{it}")
                    nc.scalar.activation(eqT[:], qT_sb[:], EXPF)
                    sumT = opool.tile([128, NCH], F32, tag="sumT",
                                      name=f"sumT_{_rep}_{it}")
                    nc.vector.reduce_sum(
                        sumT[:],
                        eqT[:].rearrange("r (ch c) -> r ch c", c=C),
                        axis=mybir.AxisListType.X,
                    )
                    rsT = opool.tile([128, NCH], F32, tag="rsT",
                                     name=f"rsT_{_rep}_{it}")
                    nc.vector.reciprocal(rsT[:], sumT[:])
                    qT_bf = opool.tile([128, NCH * C], FP8, tag="qTbf",
                                       name=f"qTbf_{_rep}_{it}")
                    nc.vector.tensor_mul(
                        qT_bf[:].rearrange("r (ch c) -> r ch c", c=C),
                        eqT[:].rearrange("r (ch c) -> r ch c", c=C),
                        rsT[:].broadcast_to([128, NCH, C]),
                    )
                    # publish local slice (pixel-major [1568, 21] bf16)
                    nc.sync.dma_start(
                        qT_sl[0 : 1536 * C].rearrange(
                            "(ch r c) -> r ch c", r=128, c=C
                        ),
                        qT_bf[:, 0 : (NCH - 1) * C].rearrange(
                            "r (ch c) -> r ch c", c=C
                        ),
                    )
                    nc.sync.dma_start(
                        qT_sl[1536 * C : COLS * C].rearrange("(r c) -> r c", c=C),
                        qT_bf[0:32, (NCH - 1) * C : NCH * C],
                    )
                    qT_full = dpool.tile(
                        [N * C], FP8, tag="qtfull", bufs=2,
                        addr_space="Shared", name=f"qtfull_{_rep}_{it}",
                    )
                    nc.gpsimd.collective_compute(
                        "AllGather",
                        mybir.AluOpType.bypass,
                        replica_groups=[list(range(NCORES))],
                        ins=[qT_sl[:]],
                        outs=[qT_full[:]],
                    )
                    # PE warmers: HAM drops the PE clock to 1.2GHz after
                    # ~3.4us idle; keep the array ticking through the whole
                    # gather gap with a chain of throwaway fp32 matmuls so
                    # the next iteration's bilateral starts at 2.4GHz.
                    warm_ps = pspool.tile([128, 512], F32, tag="d2a",
                                          name=f"warm_{_rep}_{it}")
                    for wi in range(13):
                        mm(warm_ps[:, 0:512],
                           u_sb[:, 0:128], u_sb[:, 0:512],
                           start=True, stop=True)
                else:
                    # ---- final: Q = u + aw42^T @ so42, class-major out
                    q_sb = smpool.tile([C, COLS], F32, tag="eq",
                                      name=f"qsb_{_rep}")
                    for ci, (c0, cw) in enumerate(CTS):
                        q_ps = pspool.tile([C, 512], F32,
                                           tag=("tail" if ci % 2 == 0 else "bil"),
                                           name=f"qps_{_rep}_{ci}")
                        mm(q_ps[:, 0:cw], aw54_sb[:], so54[:, c0 : c0 + cw],
                           start=True, stop=True)
                        nc.vector.tensor_add(
                            q_sb[:, c0 : c0 + cw], q_ps[:, 0:cw],
                            u_sb[:, c0 : c0 + cw]
                        )
                    nc.sync.dma_start(qt_out[:], q_sb[:])

    nc.compile()
    return nc


def _host_inputs(unaries, rgb, spatial_kernel, bilateral_kernel, compatibility_matrix):
    bf = ml_dtypes.bfloat16
    f8 = ml_dtypes.float8_e4m3fn
    u = np.transpose(np.asarray(unaries, dtype=np.float32)[0], (2, 0, 1)).reshape(C, N)
    rgbf = np.asarray(rgb, dtype=np.float32)[0].reshape(N, 3)

    yy, xx = np.meshgrid(
        np.arange(H, dtype=np.float64), np.arange(W, dtype=np.float64), indexing="ij"
    )
    pos = np.stack([xx.ravel(), yy.ravel()], axis=1)  # [N, 2] (x, y)

    fb = np.concatenate(
        [pos / THETA_ALPHA, rgbf.astype(np.float64) / THETA_BETA], axis=1
    )
    fb -= fb.mean(axis=0, keepdims=True)  # centering: reduces cancellation
    fb *= np.sqrt(ASCH)  # Schraudolph scale folded into the features
    a16 = fb.astype(bf)
    b16 = (fb - a16.astype(np.float64)).astype(bf)
    sq = (fb * fb).sum(axis=1)
    mh = -0.5 * sq + BSCH / 2  # each side carries half the exponent bias
    nh = mh.astype(bf)
    nl = (mh - nh.astype(np.float64)).astype(bf)
    one = np.ones(N, bf)

    # out[i,j] = a_i.a_j + b_i.a_j + a_i.b_j + (nh+nl)_i + (nh+nl)_j ~ -0.5 d2
    ubT = np.empty((KD, N), bf)
    ubT[0:5] = a16.T
    ubT[5:10] = b16.T
    ubT[10:15] = a16.T
    ubT[15] = nh
    ubT[16] = nl
    ubT[17] = one
    ubT[18] = one
    vbT = np.empty((KD, N), bf)
    vbT[0:5] = a16.T
    vbT[5:10] = a16.T
    vbT[10:15] = b16.T
    vbT[15] = one
    vbT[16] = one
    vbT[17] = nh
    vbT[18] = nl

    # lhsT blocks use the permutation p = 98*r + i: block-major [KD, NB, 128]
    ubT_b = ubT.reshape(KD, 128, NB).transpose(0, 2, 1)  # [KD, NB, 128]
    # row-tiled pair layout: rows 0:19 even block, rows 32:51 odd block
    ubT_d = np.zeros((64, (NB // 2) * 128), bf)
    ub3 = ubT_d.reshape(64, NB // 2, 128)
    ub3[0:KD, :, :] = ubT_b[:, 0::2, :]
    ub3[32 : 32 + KD, :, :] = ubT_b[:, 1::2, :]

    d = np.arange(-(H - 1), H, dtype=np.float64)
    g1tab = np.exp(-(d * d) / (2.0 * THETA_GAMMA**2))

    def g1(dd):
        return g1tab[np.asarray(dd) + (H - 1)]

    gx = g1(np.arange(W)[:, None] - np.arange(W)[None, :])  # [x, x']
    s1 = np.array([g1(np.arange(H) - t).sum() for t in range(H)])  # exact ns factors
    g2d_np = (gx / s1[None, :]).astype(bf)  # 1/ns x-factor folded into columns

    comp = np.asarray(compatibility_matrix, dtype=np.float64)
    A_s = -(comp @ np.asarray(spatial_kernel, dtype=np.float64))
    A_b = -(comp @ np.asarray(bilateral_kernel, dtype=np.float64))
    oh21_np = np.zeros((32, 1), np.float32)
    oh21_np[21, 0] = 1.0
    aw54_np = np.zeros((54, C), bf)
    aw54_np[0:21] = A_s.T.astype(bf)
    aw54_np[32:53] = A_b.T.astype(bf)

    # iteration-0 softmax(u), block-major bf16 + image bf16
    um = u.astype(np.float64)
    sm0 = np.exp(um - um.max(axis=0))
    sm0 /= sm0.sum(axis=0)
    sm0T = sm0.T  # [N, C] pixel-major
    sm0b_np = np.ascontiguousarray(
        sm0T.reshape(128, NB, C).astype(bf).reshape(128, NB * C)
    )
    sm0i_np = np.ascontiguousarray(sm0T.astype(bf).reshape(H, W * C))

    in_maps = []
    for c in range(NCORES):
        sl = slice(c * COLS, (c + 1) * COLS)
        dy = np.arange(H)[:, None] - (YPC * c + np.arange(YPC))[None, :]  # [y, k]
        # 1/ns y-factor folded into gy2 columns (fp8 to match the fp8 smi rhs)
        gy2_np = np.ascontiguousarray(
            (g1(dy) / s1[YPC * c + np.arange(YPC)][None, :]).astype(f8)
        )  # [112, 14]
        u_band = u[:, sl]  # [C, 1568] local (k x) pixel order
        uT_np = np.zeros((128, NCH * C), np.float32)
        ub_T = u_band.T  # [1568, C]
        for ch in range(NCH):
            pw = 128 if ch < NCH - 1 else 32
            uT_np[0:pw, ch * C : (ch + 1) * C] = ub_T[ch * 128 : ch * 128 + pw]
        in_maps.append(
            dict(
                ubT=ubT_d,
                vbT_sl=np.ascontiguousarray(
                    np.vstack([
                        vbT[:, sl], np.zeros((32 - KD, COLS), bf),
                        vbT[:, sl], np.zeros((32 - KD, COLS), bf),
                    ])
                ),
                g2d=g2d_np,
                gy2=gy2_np,
                u_sl=np.ascontiguousarray(u_band.astype(np.float32)),
                uT_d=uT_np,
                sm0b_d=sm0b_np,
                sm0i_d=sm0i_np,
                aw54=aw54_np,
                oh21=oh21_np,
            )
        )
    return in_maps


def run(inputs, trace=False, reps=1, **spmd_kwargs):
    in_maps = _host_inputs(**inputs)
    key = ("nc", reps)
    if key not in _CACHE:
        _CACHE[key] = _build_program(reps)
    nc = _CACHE[key]
    res = run_bass_kernel_spmd(
        nc, in_maps, core_ids=list(range(NCORES)), trace=trace, **spmd_kwargs
    )
    qs = [np.asarray(res.results[c]["qt_out"]) for c in range(NCORES)]
    Q = np.concatenate(qs, axis=1)  # [C, N]
    out = Q.reshape(C, H, W).transpose(1, 2, 0)[None].astype(np.float32)
    return out, res


def kernel(unaries, rgb, spatial_kernel, bilateral_kernel, compatibility_matrix):
    out, _ = run(
        dict(
            unaries=unaries,
            rgb=rgb,
            spatial_kernel=spatial_kernel,
            bilateral_kernel=bilateral_kernel,
            compatibility_matrix=compatibility_matrix,
        )
    )
    return out

